# revision 16
# baseline (speedup 1.0000x reference)
"""Additive (Bahdanau) attention scoring kernel for Trainium2, 8-core SPMD.

Reference computation (B=16, S=4096, D=1024, all fp32):
    q      = target @ Wq.T                    # [B, D]
    k      = memory @ Wk.T                    # [B, S, D]
    scores = tanh(q[:, None, :] + k) @ v      # [B, S]
    out    = softmax(scores - 1e9 * mask, axis=-1)

Sharding: batch across the 8 cores (2 batches per core), weights replicated.

Host-side prep: memory is transposed and its columns compacted to just the
unmasked positions (padded with duplicates of the first kept column to a
128-multiple). Masked positions contribute exactly 0 to the reference
softmax (exp(-1e9) == 0 in fp32), so skipping their columns is exact.
Large operands ship in bf16 — the kernel's internal matmul precision (max
rel err ~3e-3 vs the 2e-2 gate). memC is partition-major ([P, DC, s]) so a
whole batch loads with ONE dma_start of 128 x 34.8KB descriptors — the
sync-sequencer's ~0.6us-per-issue cost was the previous bottleneck.
The kernel emits compact softmax rows; the host scatters them back to
full-S positions (pad columns get -1e9 added on device, so they contribute
exp(-1e9)=0 to the softmax sum and are then discarded).

Device pipeline ([e, s] layout, everything on the PE + ACT):
  - k^T tile [e=128, s<=512] accumulates over dc: stationary = WkT chunk,
    moving = resident mem batch slice. dc-outer over half the e-tiles
    (4 PSUM banks) so the PE tracks the weight DMAs during the prologue.
  - q^T is computed directly in bias layout: per e-tile [128, 2] psum via
    Wq-chunk-stationary x target moving (64 tiny MMs, no DRAM bounce);
    the q-add is then FREE inside the ACT tanh (per-partition bias).
  - v-dot on the PE at full width: stationary V_et = v-chunk broadcast
    across 128 columns (every output row equals the v-dot), moving = tanh
    tile; skinny M=1 matmuls measured 306ns vs 216ns full-width. The pad
    mask row is added by one K=1 matmul into the same PSUM group; ACT Exp
    reads psum row 0 with accum_out producing the softmax sum for free.
  - Finale per batch: DVE reduce + reciprocal, one ACT Copy(scale=1/sum)
    over the compact row, single-descriptor DMA out.
"""

import numpy as np
import ml_dtypes

from contextlib import ExitStack

import concourse.tile as tile
from concourse import bacc, mybir

B, S, D = 16, 4096, 1024
N_CORES = 8
NB = B // N_CORES  # batches per core
P = 128
DC = D // P        # contraction chunks (8)
ET = D // P        # e tiles (8)
SW = 512           # substrip width (PSUM bank limit at fp32)

F32 = mybir.dt.float32
BF16 = mybir.dt.bfloat16
AF = mybir.ActivationFunctionType

BF16NP = ml_dtypes.bfloat16

_CACHE = {}


def substrips(s_pad):
    widths = [SW] * (s_pad // SW)
    if s_pad % SW:
        widths.append(s_pad % SW)
    return widths


def _build_program(s_pad):
    widths = substrips(s_pad)
    nsub = len(widths)

    nc = bacc.Bacc("TRN2", target_bir_lowering=False, debug=False)

    memC = nc.dram_tensor("memC", [NB, P, DC, s_pad], BF16, kind="ExternalInput").ap()
    wkT = nc.dram_tensor("wkT", [DC * P, D], BF16, kind="ExternalInput").ap()
    wqT = nc.dram_tensor("wqT", [DC * P, D], BF16, kind="ExternalInput").ap()
    tgtT = nc.dram_tensor("tgtT", [P, DC * NB], BF16, kind="ExternalInput").ap()
    vT = nc.dram_tensor("vT", [P, ET], F32, kind="ExternalInput").ap()
    mneg = nc.dram_tensor("mneg", [NB, s_pad], BF16, kind="ExternalInput").ap()
    out = nc.dram_tensor("out", [NB, s_pad], F32, kind="ExternalOutput").ap()

    with tile.TileContext(nc) as tc, ExitStack() as ctx:
        consts = ctx.enter_context(tc.tile_pool(name="consts", bufs=1))
        mb_pool = ctx.enter_context(tc.tile_pool(name="mb", bufs=2))
        th_pool = ctx.enter_context(tc.tile_pool(name="th", bufs=4))
        fin_pool = ctx.enter_context(tc.tile_pool(name="fin", bufs=2))
        kps_pool = ctx.enter_context(tc.tile_pool(name="kps", bufs=5, space="PSUM"))
        vd_pool = ctx.enter_context(tc.tile_pool(name="vd", bufs=2, space="PSUM"))
        sm_pool = ctx.enter_context(tc.tile_pool(name="smps", bufs=1, space="PSUM"))

        # --- DMA issue order is the prologue critical path (sync queue:
        # wk/sub0/wq trickled, then the rest of batch 0 per substrip, then
        # batch 1; tiny consts go via the idle gpsimd sequencer in
        # parallel). ---
        wkb = consts.tile([P, DC * D], BF16)
        membs = [mb_pool.tile([P, DC, s_pad], BF16, tag="memb", name=f"memb{b}")
                 for b in range(NB)]
        w0 = widths[0]
        wqb = consts.tile([P, DC * D], BF16)
        # trickle wk / batch-0-substrip-0 / wq together: the PE's dc-outer
        # first half consumes wk+mem dc-pairs as they land, and the q
        # matmuls (right after half 0) find wq already resident
        for dcp in range(4):
            for dc in (2 * dcp, 2 * dcp + 1):
                nc.sync.dma_start(wkb[:, dc * D:(dc + 1) * D], wkT[dc * P:(dc + 1) * P, :])
            nc.sync.dma_start(membs[0][:, 2 * dcp:2 * dcp + 2, 0:w0],
                              memC[0, :, 2 * dcp:2 * dcp + 2, 0:w0])
            for dc in (2 * dcp, 2 * dcp + 1):
                nc.sync.dma_start(wqb[:, dc * D:(dc + 1) * D], wqT[dc * P:(dc + 1) * P, :])
        # rest of batch 0 in per-substrip pieces so each substrip unblocks
        # as its own slice lands, then batch 1 whole
        off = w0
        for wnext in widths[1:]:
            nc.sync.dma_start(membs[0][:, :, off:off + wnext],
                              memC[0, :, :, off:off + wnext])
            off += wnext
        nc.sync.dma_start(membs[1][:, :, :], memC[1, :, :, :])

        tgt16 = consts.tile([P, DC * NB], BF16)
        nc.gpsimd.dma_start(tgt16[:], tgtT[:, :])
        v_sb = consts.tile([P, ET], F32)
        nc.gpsimd.dma_start(v_sb[:], vT[:, :])
        mneg_sb = consts.tile([1, NB * s_pad], BF16)
        for b in range(NB):
            nc.gpsimd.dma_start(mneg_sb[:, b * s_pad:(b + 1) * s_pad], mneg[b:b + 1, :])
        one1p = consts.tile([1, P], BF16)
        nc.vector.memset(one1p[:], 1.0)
        ones128 = consts.tile([P, P], BF16)
        nc.vector.memset(ones128[:], 1.0)
        # V_et = v chunk broadcast across 128 columns (per-partition scalar
        # broadcast along the free dim — a native DVE tensor_scalar op)
        V_all = consts.tile([P, ET * P], BF16)
        for et in range(ET):
            nc.vector.tensor_scalar_mul(V_all[:, et * P:(et + 1) * P], ones128[:],
                                        v_sb[:, et:et + 1])

        q_sb = consts.tile([P, NB * ET], F32)
        scores = [consts.tile([1, s_pad], F32, tag=f"str{b}", name=f"str{b}")
                  for b in range(NB)]
        accs = [consts.tile([1, nsub], F32, tag=f"acc{b}", name=f"acc{b}")
                for b in range(NB)]

        def emit_q():
            # q directly in bias layout: per e-tile, stationary = Wq chunk
            # [128, 128], moving = target columns [128, NB] -> [128, NB] psum
            for et in range(ET):
                q_ps = sm_pool.tile([P, NB], F32, tag="small", name="q_ps")
                for dc in range(DC):
                    nc.tensor.matmul(
                        q_ps[:],
                        wqb[:, dc * D + et * P: dc * D + (et + 1) * P],
                        tgt16[:, dc * NB:(dc + 1) * NB],
                        start=(dc == 0),
                        stop=(dc == DC - 1),
                    )
                # bias layout: q_sb[:, b*ET + et]
                for b in range(NB):
                    nc.vector.tensor_copy(q_sb[:, b * ET + et: b * ET + et + 1],
                                          q_ps[:, b:b + 1])

        first = True
        for b in range(NB):
            off = 0
            for sp, w in enumerate(widths):
                vd_ps = vd_pool.tile([P, SW], F32, tag="vd", name="vd_ps")
                ths = {}
                for half in range(2):
                    ets = range(half * 4, half * 4 + 4)
                    k_ps = {et: kps_pool.tile([P, SW], F32, tag="k", name="k_ps")
                            for et in ets}
                    for dc in range(DC):
                        for et in ets:
                            nc.tensor.matmul(
                                k_ps[et][:, :w],
                                wkb[:, dc * D + et * P: dc * D + (et + 1) * P],
                                membs[b][:, dc, off:off + w],
                                start=(dc == 0),
                                stop=(dc == DC - 1),
                            )
                    if first:
                        emit_q()
                        first = False
                    for et in ets:
                        th = th_pool.tile([P, SW], BF16, tag="th", name="th")
                        nc.scalar.activation(
                            th[:, :w], k_ps[et][:, :w], AF.Tanh,
                            bias=q_sb[:, b * ET + et: b * ET + et + 1],
                        )
                        ths[et] = th
                        # v-dot trails the tanh stream on the PE (full-width
                        # stationary: every output row equals the v-dot row)
                        if et >= 2:
                            lag = et - 2
                            nc.tensor.matmul(
                                vd_ps[:, :w], V_all[:, lag * P:(lag + 1) * P],
                                ths[lag][:, :w], start=(lag == 0), stop=False,
                            )
                for et in (ET - 2, ET - 1):
                    nc.tensor.matmul(
                        vd_ps[:, :w], V_all[:, et * P:(et + 1) * P], ths[et][:, :w],
                        start=False, stop=False,
                    )
                nc.tensor.matmul(
                    vd_ps[:, :w], one1p[:],
                    mneg_sb[:, b * s_pad + off: b * s_pad + off + w],
                    start=False, stop=True,
                )
                nc.scalar.activation(
                    scores[b][:, off:off + w], vd_ps[0:1, :w], AF.Exp,
                    accum_out=accs[b][:, sp:sp + 1],
                )
                off += w
            # softmax normalization for this batch (compact row) — emitted
            # here so batch 0's finale overlaps batch 1's compute
            tot = fin_pool.tile([1, 1], F32, tag="tot", name="tot")
            nc.vector.reduce_sum(tot[:], accs[b][:], axis=mybir.AxisListType.X)
            recip = fin_pool.tile([1, 1], F32, tag="recip", name="recip")
            nc.vector.reciprocal(recip[:], tot[:])
            outs = fin_pool.tile([1, s_pad], F32, tag="outs", name="outs")
            # scale split across ACT and DVE so the serial tail is shorter
            cut = (3 * s_pad // 4) // P * P
            nc.scalar.activation(outs[:, :cut], scores[b][:, :cut], AF.Copy,
                                 scale=recip[:, 0:1])
            nc.vector.tensor_scalar_mul(outs[:, cut:], scores[b][:, cut:],
                                        recip[:, 0:1])
            nc.sync.dma_start(out[b:b + 1, :], outs[:])

    nc.compile()
    return nc


def get_program(s_pad=None):
    assert s_pad is not None
    if s_pad not in _CACHE:
        _CACHE[s_pad] = _build_program(s_pad)
    return _CACHE[s_pad]


def prepare_in_maps(memory, target, memory_mask, Wq, Wk, v):
    memory = np.asarray(memory, dtype=np.float32)
    target = np.asarray(target, dtype=np.float32)
    Wq = np.asarray(Wq, dtype=np.float32)
    Wk = np.asarray(Wk, dtype=np.float32)
    v = np.asarray(v, dtype=np.float32)
    mask = np.asarray(memory_mask)

    keep_bool = ~mask                                                # [B, S]
    max_kept = int(keep_bool.sum(1).max())
    s_pad = max(512, ((max_kept + 127) // 128) * 128)

    memT = memory.transpose(0, 2, 1)                                 # [B, D, S] view
    kept_pad = np.empty((B, s_pad), dtype=np.int64)
    kept_count = np.empty(B, dtype=np.int64)
    for b in range(B):
        k = np.flatnonzero(keep_bool[b])
        kept_count[b] = len(k)
        kept_pad[b, :len(k)] = k
        kept_pad[b, len(k):] = k[0]  # pad data: duplicate first kept column
    # partition-major compact memory: memC[b, p, dc, s] = memory[b, kept[s], dc*128+p]
    memC = np.empty((B, D, s_pad), dtype=BF16NP)
    for b in range(B):
        memC[b] = memT[b][:, kept_pad[b]]
    memC = np.ascontiguousarray(
        memC.reshape(B, DC, P, s_pad).transpose(0, 2, 1, 3))         # [B, P, DC, s]

    # compact pad mask: 0 at kept positions, -1e9 at pads (pads then produce
    # exp(-1e9) == 0 and never pollute the softmax sum)
    mnegC = np.zeros((B, s_pad), dtype=np.float32)
    for b in range(B):
        mnegC[b, kept_count[b]:] = -1e9
    mnegC = mnegC.astype(BF16NP)

    wkT = np.ascontiguousarray(Wk.T).astype(BF16NP)                  # [D, D]
    wqT = np.ascontiguousarray(Wq.T).astype(BF16NP)                  # [D, D]
    # bias-layout target: [P, DC*B] with column dc*B+b = target[b, dc*128:* ]
    tgtT = np.ascontiguousarray(
        target.T.reshape(DC, P, B).transpose(1, 0, 2).reshape(P, DC * B)
    ).astype(BF16NP)
    vTh = np.ascontiguousarray(v.reshape(ET, P).T)                   # [P, ET] fp32

    in_maps = [
        {
            "memC": np.ascontiguousarray(memC[c * NB:(c + 1) * NB]),
            "wkT": wkT,
            "wqT": wqT,
            "tgtT": np.ascontiguousarray(
                tgtT.reshape(P, DC, B)[:, :, c * NB:(c + 1) * NB].reshape(P, DC * NB)),
            "vT": vTh,
            "mneg": np.ascontiguousarray(mnegC[c * NB:(c + 1) * NB]),
        }
        for c in range(N_CORES)
    ]
    global _LAST_META
    _LAST_META = (kept_pad, kept_count, s_pad)
    return in_maps, s_pad


_LAST_META = None


def gather_output(results, meta=None):
    kept_pad, kept_count, s_pad = meta if meta is not None else _LAST_META
    out = np.zeros((B, S), dtype=np.float32)
    for c in range(N_CORES):
        vals = results[c]["out"].reshape(NB, s_pad)
        for bb in range(NB):
            b = c * NB + bb
            kc = kept_count[b]
            out[b, kept_pad[b, :kc]] = vals[bb, :kc]
    return out


def kernel(memory, target, memory_mask, Wq, Wk, v):
    from concourse.bass_utils import run_bass_kernel_spmd

    in_maps, s_pad = prepare_in_maps(memory, target, memory_mask, Wq, Wk, v)
    nc = get_program(s_pad=s_pad)
    res = run_bass_kernel_spmd(nc, in_maps, list(range(N_CORES)))
    return gather_output(res.results)
